# revision 8
# baseline (speedup 1.0000x reference)
"""SE(3) compose-scan Trainium2 kernel (nn_ComposeRt).

x [131072, 32, 3, 4] fp32 -> cumulative compose along axis 1:
out[b,0] = x[b,0]; out[b,n] = out[b,n-1] o x[b,n],
[rA|tA] o [rB|tB] = [rA@rB | tA + rA@tB].

Sharding: pure data parallel over batch across 8 NeuronCores.
Per core: batch b_local = p*F + f (partition p, slot f).

Numerics: fp16 on device with homogeneous prescaling. Host scales every
x by s = 3^-0.5 (all 12 entries). Treating each x as the top rows of a
4x4 with bottom row (0,0,0,1), the scaled chain uses bottom-right s, so
the device recurrence is rot = rA@rB, trans = s*tA + rA@tB, and the
stored carry is exactly s^(n+1) * out_n. The host multiplies 3^((n+1)/2)
back into the fp32 result. Values stay O(100) -- far from fp16 limits --
and full-batch simulated rel err vs f64 is 1.9e-3 (gate 2e-2).

Performance: tiles are laid out [P, n, 3(row), 4(col), F] with the
batch-slot dim f innermost (stride 1, count 128). Every DVE op then has
a packed 16-bit innermost dim, so tensor_tensor runs in 2x_1P mode
(2 elem/cycle) -- the rot-product broadcasts sit on middle AP dims and
no longer block packing.

Engine split: the DVE runs the rotation chain (3 muls + 2 adds per
step; it never reads column 3 of the carry). The translation column
runs as a separate chain on GpSimd: tau_n = s*tau_{n-1} + (rA@tB)_n,
reading column 3 of the DVE's output tiles one step behind. tau is
DMA'd out per block and the host stitches it in as column 3 (the C
tiles' own column 3 holds rA@tB, which is discarded).
"""

import sys

if "/opt/trn_rl_repo" not in sys.path:
    sys.path.insert(0, "/opt/trn_rl_repo")

import numpy as np

import concourse.bacc as bacc
import concourse.mybir as mybir
from concourse import bass_utils
from concourse.tile import TileContext

P = 128
N = 32
N_CORES = 8
B = 131072

F = 128  # batch slots per partition
NSUB = 2  # n per DMA block
HALVES = N // NSUB
B_CORE = P * F
assert B_CORE * N_CORES == B

SCALE = float(1.0 / np.sqrt(np.float64(3.0)))

BLK = NSUB * 12 * F  # elems per DMA block per partition
TBLK = NSUB * 3 * F  # translation elems per block per partition

TRANS_ENGINE = "gpsimd"  # "gpsimd" (2 TT ops) | "dve" (1 STT op)


def build():
    nc = bacc.Bacc("TRN2", target_bir_lowering=False, debug=False)
    x = nc.dram_tensor("x", [HALVES, P, BLK], mybir.dt.float16, kind="ExternalInput")
    y = nc.dram_tensor("y", [HALVES, P, BLK], mybir.dt.float16, kind="ExternalOutput")
    yt = nc.dram_tensor(
        "yt", [HALVES, P, TBLK], mybir.dt.float16, kind="ExternalOutput"
    )

    with TileContext(nc) as tc:
        with (
            tc.tile_pool(name="xin", bufs=3) as xpool,
            tc.tile_pool(name="outp", bufs=4) as opool,
            tc.tile_pool(name="work", bufs=2) as wpool,
            tc.tile_pool(name="trans", bufs=3) as tpool,
            tc.tile_pool(name="const", bufs=1) as cpool,
        ):
            st = cpool.tile([P, 1], mybir.dt.float16, tag="s")
            nc.gpsimd.memset(st[:], SCALE)
            s_bc3 = st.unsqueeze(1).broadcast_to([P, 3, F])
            prev = None  # [P, 3, 4, F] rot carry view (cols 0..2 valid)
            prev_tau = None  # [P, 3, F] translation carry view
            for h in range(HALVES):
                xt = xpool.tile([P, BLK], mybir.dt.float16, tag="x")
                nc.sync.dma_start(out=xt[:], in_=x.ap()[h])
                ot = opool.tile([P, BLK], mybir.dt.float16, tag="o")
                tt = tpool.tile([P, TBLK], mybir.dt.float16, tag="t")
                xv = xt.rearrange("p (n i j f) -> p n i j f", n=NSUB, i=3, j=4)
                ov = ot.rearrange("p (n i j f) -> p n i j f", n=NSUB, i=3, j=4)
                of = ot.rearrange("p (n e) -> p n e", n=NSUB)
                tv3 = tt.rearrange("p (n i f) -> p n i f", n=NSUB, i=3)
                for nl in range(NSUB):
                    Bm = xv[:, nl]  # [P, 3, 4, F]
                    Cm = ov[:, nl]
                    tau = tv3[:, nl]
                    if h == 0 and nl == 0:
                        # chain starts at x_0 itself; no copy needed.
                        nc.vector.tensor_copy(out=tau, in_=xv[:, 0, :, 3, :])
                        prev = xv[:, 0]
                        prev_tau = tau
                        continue
                    A = prev
                    tw = wpool.tile([P, 12 * F], mybir.dt.float16, tag="tv")
                    twv = tw.rearrange("p (i j f) -> p i j f", i=3, j=4)
                    sh = [P, 3, 4, F]
                    # C = sum_k A[:, i, k, f] * B[:, k, j, f]
                    for k in range(3):
                        a_op = A[:, :, k, :].unsqueeze(2).broadcast_to(sh)
                        b_op = Bm[:, k].unsqueeze(1).broadcast_to(sh)
                        if k == 0:
                            nc.vector.tensor_mul(out=Cm, in0=a_op, in1=b_op)
                        else:
                            nc.vector.tensor_mul(out=twv, in0=a_op, in1=b_op)
                            nc.vector.tensor_add(
                                out=of[:, nl], in0=of[:, nl], in1=tw[:]
                            )
                    # translation chain: tau = s*prev_tau + C[:,:,3,:]
                    if TRANS_ENGINE == "gpsimd":
                        nc.gpsimd.tensor_mul(out=tau, in0=prev_tau, in1=s_bc3)
                        nc.gpsimd.tensor_add(
                            out=tau, in0=tau, in1=Cm[:, :, 3, :]
                        )
                    else:
                        nc.vector.scalar_tensor_tensor(
                            out=tau,
                            in0=prev_tau,
                            scalar=SCALE,
                            in1=Cm[:, :, 3, :],
                            op0=mybir.AluOpType.mult,
                            op1=mybir.AluOpType.add,
                        )
                    prev = Cm
                    prev_tau = tau
                if h == 0:
                    # n=0 output is x_0 itself; the host fills it from the
                    # input, so only the n=1 slab is written.
                    nc.sync.dma_start(
                        out=y.ap()[0][:, 12 * F : 24 * F], in_=of[:, 1]
                    )
                else:
                    nc.sync.dma_start(out=y.ap()[h], in_=ot[:])
                nc.sync.dma_start(out=yt.ap()[h], in_=tt[:])
    nc.compile()
    return nc


_NC_CACHE = []


def _get_nc():
    if not _NC_CACHE:
        _NC_CACHE.append(build())
    return _NC_CACHE[0]


def shard_input(x_full):
    """x_full: [B, N, 12] fp32 -> per-core [HALVES, P, BLK] fp16, scaled."""
    xs = (x_full * np.float32(SCALE)).astype(np.float16)
    out = []
    for c in range(N_CORES):
        xc = xs[c * B_CORE : (c + 1) * B_CORE].reshape(P, F, HALVES, NSUB, 12)
        xc = np.ascontiguousarray(xc.transpose(2, 0, 3, 4, 1))  # h p n e f
        out.append(xc.reshape(HALVES, P, BLK))
    return out


def unshard_output(ys, yts, x_full):
    """ys: per-core [HALVES, P, BLK]; yts: per-core [HALVES, P, TBLK].
    x_full only supplies out[:, 0] = x[:, 0] (identity first element)."""
    parts = []
    for c in range(N_CORES):
        a = ys[c].reshape(HALVES, P, NSUB, 3, 4, F)
        a = a.transpose(1, 5, 0, 2, 3, 4).reshape(B_CORE, N, 3, 4)
        t = yts[c].reshape(HALVES, P, NSUB, 3, F)
        t = t.transpose(1, 4, 0, 2, 3).reshape(B_CORE, N, 3)
        a[:, :, :, 3] = t
        parts.append(a)
    out = np.concatenate(parts, axis=0).astype(np.float32)
    fac = (np.float64(3.0) ** ((np.arange(N) + 1) / 2.0)).astype(np.float32)
    out *= fac[None, :, None, None]
    # out[:, 0] = x[:, 0] exactly (kernel never writes the n=0 slab of y).
    out[:, 0] = x_full.reshape(B, N, 3, 4)[:, 0]
    return out


def run(x, trace=False, trace_kwargs=None):
    """Returns (out [B,N,3,4], BassKernelResults)."""
    x = np.asarray(x, dtype=np.float32).reshape(B, N, 12)
    nc = _get_nc()
    in_maps = [{"x": xc} for xc in shard_input(x)]
    res = bass_utils.run_bass_kernel_spmd(
        nc,
        in_maps,
        list(range(N_CORES)),
        trace=trace,
        **(trace_kwargs or {}),
    )
    out = unshard_output(
        [r["y"] for r in res.results], [r["yt"] for r in res.results], x
    )
    return out.reshape(B, N, 3, 4), res


def kernel(x):
    return run(x)[0]


# revision 13
# speedup vs baseline: 1.1433x; 1.1433x over previous
"""SE(3) compose-scan Trainium2 kernel (nn_ComposeRt).

x [131072, 32, 3, 4] fp32 -> cumulative compose along axis 1:
out[b,0] = x[b,0]; out[b,n] = out[b,n-1] o x[b,n],
[rA|tA] o [rB|tB] = [rA@rB | tA + rA@tB].

Sharding: pure data parallel over batch across 8 NeuronCores.
Per core: batch b_local = p*F + f (partition p, slot f).

Numerics: fp16 on device with homogeneous prescaling. Host scales every
x by s = 3^-0.5 (all 12 entries). Treating each x as the top rows of a
4x4 with bottom row (0,0,0,1), the scaled chain uses bottom-right s, so
the device recurrence is rot = rA@rB, trans = s*tA + rA@tB, and the
stored carry is exactly s^(n+1) * out_n. The host multiplies 3^((n+1)/2)
back into the fp32 result. Values stay O(100) -- far from fp16 limits --
and full-batch simulated rel err vs f64 is 1.9e-3 (gate 2e-2).

Performance: tiles are laid out [P, n, 3(row), 4(col), F] with the
batch-slot dim f innermost (stride 1, count 128). Every DVE op then has
a packed 16-bit innermost dim, so tensor_tensor runs in 2x_1P mode
(2 elem/cycle) -- the rot-product broadcasts sit on middle AP dims and
no longer block packing.

Engine split: the DVE runs the rotation chain (3 muls + 2 adds per
step; it never reads column 3 of the carry). The translation column
runs as a separate chain on GpSimd: tau_n = s*tau_{n-1} + (rA@tB)_n,
reading column 3 of the DVE's output tiles one step behind. tau is
DMA'd out per block and the host stitches it in as column 3 (the C
tiles' own column 3 holds rA@tB, which is discarded).
"""

import sys

if "/opt/trn_rl_repo" not in sys.path:
    sys.path.insert(0, "/opt/trn_rl_repo")

import numpy as np

import concourse.bacc as bacc
import concourse.mybir as mybir
from concourse import bass_utils
from concourse.tile import TileContext

P = 128
N = 32
N_CORES = 8
B = 131072

F = 128  # batch slots per partition
NSUB = 2  # n per DMA block
HALVES = N // NSUB
B_CORE = P * F
assert B_CORE * N_CORES == B

SCALE = float(1.0 / np.sqrt(np.float64(3.0)))

BLK = NSUB * 12 * F  # elems per DMA block per partition
TBLK = NSUB * 3 * F  # translation elems per block per partition

# "dve": one scalar_tensor_tensor on the Vector engine. "gpsimd" (2 TT ops)
# was measured 23us SLOWER overall: concurrent GpSimd work contends for the
# shared SBUF port and inflates DVE 2x_1P ops by ~25% while it runs.
TRANS_ENGINE = "dve"


def build():
    nc = bacc.Bacc("TRN2", target_bir_lowering=False, debug=False)
    x = nc.dram_tensor("x", [HALVES, P, BLK], mybir.dt.float16, kind="ExternalInput")
    y = nc.dram_tensor("y", [HALVES, P, BLK], mybir.dt.float16, kind="ExternalOutput")
    yt = nc.dram_tensor(
        "yt", [HALVES, P, TBLK], mybir.dt.float16, kind="ExternalOutput"
    )

    with TileContext(nc) as tc:
        with (
            tc.tile_pool(name="xin", bufs=3) as xpool,
            tc.tile_pool(name="outp", bufs=4) as opool,
            tc.tile_pool(name="work", bufs=2) as wpool,
            tc.tile_pool(name="trans", bufs=3) as tpool,
            tc.tile_pool(name="const", bufs=1) as cpool,
        ):
            if TRANS_ENGINE == "gpsimd":
                st = cpool.tile([P, 1], mybir.dt.float16, tag="s")
                nc.gpsimd.memset(st[:], SCALE)
                s_bc3 = st.unsqueeze(1).broadcast_to([P, 3, F])
            prev = None  # [P, 3, 4, F] rot carry view (cols 0..2 valid)
            prev_tau = None  # [P, 3, F] translation carry view
            for h in range(HALVES):
                xt = xpool.tile([P, BLK], mybir.dt.float16, tag="x")
                if h == 0:
                    # split the first load so compute can start after the
                    # first half-block lands
                    nc.sync.dma_start(
                        out=xt[:, 0 : 12 * F], in_=x.ap()[0][:, 0 : 12 * F]
                    )
                    nc.sync.dma_start(
                        out=xt[:, 12 * F : 24 * F],
                        in_=x.ap()[0][:, 12 * F : 24 * F],
                    )
                else:
                    nc.sync.dma_start(out=xt[:], in_=x.ap()[h])
                ot = opool.tile([P, BLK], mybir.dt.float16, tag="o")
                tt = tpool.tile([P, TBLK], mybir.dt.float16, tag="t")
                xv = xt.rearrange("p (n i j f) -> p n i j f", n=NSUB, i=3, j=4)
                ov = ot.rearrange("p (n i j f) -> p n i j f", n=NSUB, i=3, j=4)
                of = ot.rearrange("p (n e) -> p n e", n=NSUB)
                tv3 = tt.rearrange("p (n i f) -> p n i f", n=NSUB, i=3)
                for nl in range(NSUB):
                    Bm = xv[:, nl]  # [P, 3, 4, F]
                    Cm = ov[:, nl]
                    tau = tv3[:, nl]
                    if h == 0 and nl == 0:
                        # chain starts at x_0 itself; no copy needed.
                        nc.vector.tensor_copy(out=tau, in_=xv[:, 0, :, 3, :])
                        prev = xv[:, 0]
                        prev_tau = tau
                        continue
                    A = prev
                    tw = wpool.tile([P, 12 * F], mybir.dt.float16, tag="tv")
                    twv = tw.rearrange("p (i j f) -> p i j f", i=3, j=4)
                    sh = [P, 3, 4, F]
                    # C = sum_k A[:, i, k, f] * B[:, k, j, f]
                    for k in range(3):
                        a_op = A[:, :, k, :].unsqueeze(2).broadcast_to(sh)
                        b_op = Bm[:, k].unsqueeze(1).broadcast_to(sh)
                        if k == 0:
                            nc.vector.tensor_mul(out=Cm, in0=a_op, in1=b_op)
                        else:
                            nc.vector.tensor_mul(out=twv, in0=a_op, in1=b_op)
                            nc.vector.tensor_add(
                                out=of[:, nl], in0=of[:, nl], in1=tw[:]
                            )
                    # translation chain: tau = s*prev_tau + C[:,:,3,:]
                    if TRANS_ENGINE == "gpsimd":
                        nc.gpsimd.tensor_mul(out=tau, in0=prev_tau, in1=s_bc3)
                        nc.gpsimd.tensor_add(
                            out=tau, in0=tau, in1=Cm[:, :, 3, :]
                        )
                    else:
                        nc.vector.scalar_tensor_tensor(
                            out=tau,
                            in0=prev_tau,
                            scalar=SCALE,
                            in1=Cm[:, :, 3, :],
                            op0=mybir.AluOpType.mult,
                            op1=mybir.AluOpType.add,
                        )
                    prev = Cm
                    prev_tau = tau
                if h == 0:
                    # n=0 output is x_0 itself; the host fills it from the
                    # input, so only the n=1 slab is written.
                    nc.sync.dma_start(
                        out=y.ap()[0][:, 12 * F : 24 * F], in_=of[:, 1]
                    )
                    nc.sync.dma_start(out=yt.ap()[h], in_=tt[:])
                elif h == HALVES - 1:
                    # split the last stores so the first half overlaps the
                    # final step's compute
                    nc.sync.dma_start(
                        out=y.ap()[h][:, 0 : 12 * F], in_=of[:, 0]
                    )
                    nc.sync.dma_start(
                        out=yt.ap()[h][:, 0 : 3 * F], in_=tv3[:, 0]
                    )
                    nc.sync.dma_start(
                        out=y.ap()[h][:, 12 * F : 24 * F], in_=of[:, 1]
                    )
                    nc.sync.dma_start(
                        out=yt.ap()[h][:, 3 * F : 6 * F], in_=tv3[:, 1]
                    )
                else:
                    nc.sync.dma_start(out=y.ap()[h], in_=ot[:])
                    nc.sync.dma_start(out=yt.ap()[h], in_=tt[:])
    nc.compile()
    return nc


_NC_CACHE = []


def _get_nc():
    if not _NC_CACHE:
        _NC_CACHE.append(build())
    return _NC_CACHE[0]


def shard_input(x_full):
    """x_full: [B, N, 12] fp32 -> per-core [HALVES, P, BLK] fp16, scaled."""
    xs = (x_full * np.float32(SCALE)).astype(np.float16)
    out = []
    for c in range(N_CORES):
        xc = xs[c * B_CORE : (c + 1) * B_CORE].reshape(P, F, HALVES, NSUB, 12)
        xc = np.ascontiguousarray(xc.transpose(2, 0, 3, 4, 1))  # h p n e f
        out.append(xc.reshape(HALVES, P, BLK))
    return out


def unshard_output(ys, yts, x_full):
    """ys: per-core [HALVES, P, BLK]; yts: per-core [HALVES, P, TBLK].
    x_full only supplies out[:, 0] = x[:, 0] (identity first element)."""
    parts = []
    for c in range(N_CORES):
        a = ys[c].reshape(HALVES, P, NSUB, 3, 4, F)
        a = a.transpose(1, 5, 0, 2, 3, 4).reshape(B_CORE, N, 3, 4)
        t = yts[c].reshape(HALVES, P, NSUB, 3, F)
        t = t.transpose(1, 4, 0, 2, 3).reshape(B_CORE, N, 3)
        a[:, :, :, 3] = t
        parts.append(a)
    out = np.concatenate(parts, axis=0).astype(np.float32)
    fac = (np.float64(3.0) ** ((np.arange(N) + 1) / 2.0)).astype(np.float32)
    out *= fac[None, :, None, None]
    # out[:, 0] = x[:, 0] exactly (kernel never writes the n=0 slab of y).
    out[:, 0] = x_full.reshape(B, N, 3, 4)[:, 0]
    return out


def run(x, trace=False, trace_kwargs=None):
    """Returns (out [B,N,3,4], BassKernelResults)."""
    x = np.asarray(x, dtype=np.float32).reshape(B, N, 12)
    nc = _get_nc()
    in_maps = [{"x": xc} for xc in shard_input(x)]
    res = bass_utils.run_bass_kernel_spmd(
        nc,
        in_maps,
        list(range(N_CORES)),
        trace=trace,
        **(trace_kwargs or {}),
    )
    out = unshard_output(
        [r["y"] for r in res.results], [r["yt"] for r in res.results], x
    )
    return out.reshape(B, N, 3, 4), res


def kernel(x):
    return run(x)[0]
